# revision 6
# baseline (speedup 1.0000x reference)
"""Trainium2 Bass kernel for the 4-term contrastive loss (N=8192, D=64).

Math: for each of the 4 similarity matrices M_k (mp2sc, sc2mp, mp2mp, sc2sc)
the loss term only needs, per row i:
    rs_k[i]  = sum_j exp(2 * cos(z_i, w_j))        (rowsum)
    num_k[i] = sum_j exp(2 * cos(z_i, w_j)) * pos[i, j]
and then  loss = -mean_i sum_k log(num_k[i] / (rs_k[i] + 1e-8)).

Sharding: each of the 8 cores owns 1024 rows and computes the row-stripes of
all four matrices tile-by-tile ([128, 2048] supertiles): PE matmul (K=64,
fp16 operands) -> PSUM; ACT exp(2s) PSUM->SBUF with fused accum_out rowsum;
one fused DVE scalar_tensor_tensor multiply with the pos tile producing the
numerator partial via accum_out. All reductions are row-wise, so there is no
cross-core communication; the host sums 8x128 per-partition partials.

pos never materializes beyond a few [128, 2048] tiles; it is streamed once
per core (16 MB as fp16 -- exact, since pos is a 0/1 mask).

z is DMAed in a p-chunk layout (partition p = consecutive rows) so the load
is one contiguous transfer; the PE transposes that produce the [K=64, n]
operand layout then yield an interleaved column order, which the host
compensates by permuting pos rows/columns identically (all reductions are
order-invariant).
"""

import numpy as np

import concourse.bass as bass
import concourse.tile as tile
from concourse import mybir
from concourse.vector_clock import ScopedClock

F32 = mybir.dt.float32
F16 = mybir.dt.float16

N = 8192
D = 64
NCORES = 8
ROWS = N // NCORES          # 1024 rows owned per core
P = 128
ITILES = ROWS // P          # 8 row-tiles per core
JSUP = 2048                 # supertile free dim (4 PSUM banks)
NJS = N // JSUP             # 4
MMN = 512                   # matmul free dim (1 PSUM bank)
NMAT = 4
TEMP_SCALE = 2.0            # 1 / TEMP with TEMP=0.5
EPS_ROW = 1e-8
EPS_NORM = 1e-12


def _patch_tile_drain():
    """This container's walrus rejects CTRL(NOP/Drain) instructions carrying
    more than ~2 sync waits; the stock TileContext exit puts one wait per
    pending vector-clock proc on a single Drain. Redistribute them one per
    NOP."""
    import concourse.tile as tile_mod

    def _drain_and_barrier_split(self, tick_clock, wait_clock):
        probe = self.nc.sync.nop(nofuse=True)
        wait_clock.add_sem_waits(
            probe.ins, ScopedClock({None: tick_clock.global_clock})
        )
        si = probe.ins.sync_info
        waits = list(si.on_wait) if si is not None and si.on_wait else []
        if len(waits) > 1:
            si.on_wait = waits[:1]
            for w in waits[1:]:
                extra = self.nc.sync.nop(nofuse=True)
                esi = extra.ins.sync_info
                if esi is None:
                    extra.ins.sync_info = mybir.SyncInfo(on_wait=[w], on_update=[])
                else:
                    esi.on_wait = [w]
        self.nc.sync.drain()
        self.nc.all_engine_barrier()
        assert self.sems is not None
        popped = self.nc._tile_sem_poison_stack.pop()
        assert popped is self._sem_poison
        self.nc.clear_and_free_semaphores(list(self.sems.allocated().values()))
        self.nc.all_engine_barrier()

    tile_mod.TileContext._drain_and_barrier = _drain_and_barrier_split


_patch_tile_drain()


def _split_excess_waits(nc, maxw=1):
    """This walrus build rejects instructions carrying more than ~1-2 sync
    waits ("Too many sync wait commands"). Hoist excess waits onto NOPs
    inserted just before the instruction on the same engine queue --
    semantically identical (engine queues execute in program order)."""
    nsplit = 0
    for bb in nc.main_func.blocks:
        out = []
        for inst in bb.instructions:
            si = getattr(inst, "sync_info", None)
            if si is not None and si.on_wait and len(si.on_wait) > maxw:
                waits = list(si.on_wait)
                for w in waits[maxw:]:
                    nop = mybir.InstNoOp(
                        name=nc.get_next_instruction_name(), ins=[], outs=[])
                    nop.engine = inst.engine
                    nop.sync_info = mybir.SyncInfo(on_wait=[w], on_update=[])
                    nc.register_instruction(nop)
                    out.append(nop)
                    nsplit += 1
                si.on_wait = waits[:maxw]
            out.append(inst)
        bb.instructions[:] = out
    return nsplit


def _normalize_and_transpose(nc, pools, ident, z_dram, nrows, blockw, tag):
    """Emit: load z [nrows, 64] f32 in p-chunk layout, l2-normalize rows,
    PE-transpose into fp16 [64, blockw] operand tiles.

    Returns nrows//blockw SBUF tiles [64, blockw] fp16. Within block b,
    column s*128 + p holds the normalized row (nrows//128)*p + (blockw//128)*b + s
    (interleaved order -- the host permutes pos to match).
    """
    ntile = nrows // P  # rows per partition
    work, stats, out_pool, pst_pool = pools

    z_nat = work.tile([P, ntile, D], F32, tag="znat")
    nc.sync.dma_start(z_nat[:], z_dram.ap().rearrange("(p t) d -> p t d", p=P))

    sq = work.tile([P, ntile, D], F32, tag="sq")
    nc.vector.tensor_tensor(out=sq[:], in0=z_nat[:], in1=z_nat[:],
                            op=mybir.AluOpType.mult)
    ss = stats.tile([P, ntile], F32, tag="ss")
    nc.vector.tensor_reduce(out=ss[:], in_=sq[:], axis=mybir.AxisListType.X,
                            op=mybir.AluOpType.add)
    # clamp ||z||^2 at EPS_NORM^2 so 1/max(||z||, eps) is exact
    nc.vector.tensor_scalar_max(out=ss[:], in0=ss[:], scalar1=EPS_NORM * EPS_NORM)
    s0 = stats.tile([P, ntile], F32, tag="s0")
    nc.scalar.activation(s0[:], ss[:], mybir.ActivationFunctionType.Sqrt)
    # one Newton step to clean up ACT sqrt: s1 = 0.5*(s0 + ss/s0); the 0.5
    # cancels inside rn = 1/s1 up to a factor folded below.
    t0 = stats.tile([P, ntile], F32, tag="t0")
    nc.vector.reciprocal(t0[:], s0[:])
    s1 = stats.tile([P, ntile], F32, tag="s1")
    nc.vector.tensor_tensor(out=s1[:], in0=ss[:], in1=t0[:],
                            op=mybir.AluOpType.mult)
    nc.vector.tensor_tensor(out=s1[:], in0=s1[:], in1=s0[:],
                            op=mybir.AluOpType.add)
    rn = stats.tile([P, ntile], F32, tag="rn")
    nc.vector.reciprocal(rn[:], s1[:])
    nc.vector.tensor_scalar_mul(out=rn[:], in0=rn[:], scalar1=2.0)

    zn = work.tile([P, ntile, D], F32, tag="zn")
    rn_b = rn[:].broadcast_to((P, ntile, D))
    nc.vector.tensor_tensor(out=zn[:], in0=z_nat[:], in1=rn_b,
                            op=mybir.AluOpType.mult)

    sub = blockw // P
    out_tiles = []
    for b in range(ntile // sub):
        ot = out_pool.tile([D, blockw], F16, tag=f"zT{tag}_{b}")
        for s in range(sub):
            t = b * sub + s
            pst = pst_pool.tile([D, P], F32, tag="pst")
            nc.tensor.transpose(pst[:], zn[:, t, :], ident[:])
            if t % 2 == 0:
                nc.scalar.copy(ot[:, s * P:(s + 1) * P], pst[:])
            else:
                nc.vector.tensor_copy(ot[:, s * P:(s + 1) * P], pst[:])
        out_tiles.append(ot)
    return out_tiles


def build_kernel():
    nc = bass.Bass("TRN2", target_bir_lowering=False, debug=False,
                   num_devices=NCORES)
    zmp = nc.dram_tensor("zmp", [N, D], F32, kind="ExternalInput")
    zsc = nc.dram_tensor("zsc", [N, D], F32, kind="ExternalInput")
    zmp_own = nc.dram_tensor("zmp_own", [ROWS, D], F32, kind="ExternalInput")
    zsc_own = nc.dram_tensor("zsc_own", [ROWS, D], F32, kind="ExternalInput")
    pos = nc.dram_tensor("pos", [ROWS, N], F16, kind="ExternalInput")
    ident = nc.dram_tensor("ident", [P, P], F32, kind="ExternalInput")
    out = nc.dram_tensor("out", [P, 1], F32, kind="ExternalOutput")

    with tile.TileContext(nc) as tc:
        with (
            tc.tile_pool(name="zops", bufs=1) as zops,
            tc.tile_pool(name="posp", bufs=3) as posp,
            tc.tile_pool(name="expp", bufs=3) as expp,
            tc.tile_pool(name="prodp", bufs=2) as prodp,
            tc.tile_pool(name="accs", bufs=1) as accs,
        ):
            ident_t = zops.tile([P, P], F32, tag="ident")
            nc.sync.dma_start(ident_t[:], ident.ap())

            with (
                tc.tile_pool(name="zwork", bufs=1) as work,
                tc.tile_pool(name="stats", bufs=2) as stats,
                tc.tile_pool(name="pstp", bufs=4, space="PSUM") as pst_pool,
            ):
                pools = (work, stats, zops, pst_pool)
                # full tensors -> 16 rhs tiles [64, 512] each
                zT_mp = _normalize_and_transpose(
                    nc, pools, ident_t, zmp, N, MMN, "mp")
                zT_sc = _normalize_and_transpose(
                    nc, pools, ident_t, zsc, N, MMN, "sc")
                # owned rows -> 8 lhsT tiles [64, 128] each
                oT_mp = _normalize_and_transpose(
                    nc, pools, ident_t, zmp_own, ROWS, P, "omp")
                oT_sc = _normalize_and_transpose(
                    nc, pools, ident_t, zsc_own, ROWS, P, "osc")

            rs_parts = accs.tile([P, NMAT * ITILES * NJS], F32, tag="rsp")
            num_parts = accs.tile([P, NMAT * ITILES * NJS], F32, tag="nump")

            mats = [(oT_mp, zT_sc), (oT_sc, zT_mp), (oT_mp, zT_mp),
                    (oT_sc, zT_sc)]

            with tc.tile_pool(name="mm", bufs=2, space="PSUM") as mm_pool:
                pos_r = pos.ap().rearrange("(t p) n -> p t n", p=P)
                for it in range(ITILES):
                    for js in range(NJS):
                        pos_t = posp.tile([P, JSUP], F16, tag="pos")
                        nc.sync.dma_start(
                            pos_t[:], pos_r[:, it, js * JSUP:(js + 1) * JSUP])
                        for m, (lt, rt) in enumerate(mats):
                            ps = mm_pool.tile([P, JSUP], F32, tag="ps")
                            for k in range(JSUP // MMN):
                                jb = js * (JSUP // MMN) + k
                                nc.tensor.matmul(
                                    ps[:, k * MMN:(k + 1) * MMN],
                                    lhsT=lt[it][:],
                                    rhs=rt[jb][:],
                                    start=True, stop=True)
                            col = (m * ITILES + it) * NJS + js
                            exp_t = expp.tile([P, JSUP], F16, tag="exp")
                            nc.scalar.activation(
                                exp_t[:], ps[:],
                                mybir.ActivationFunctionType.Exp,
                                scale=TEMP_SCALE,
                                accum_out=rs_parts[:, col:col + 1])
                            prod_t = prodp.tile([P, JSUP], F16, tag="prod")
                            nc.vector.scalar_tensor_tensor(
                                out=prod_t[:],
                                in0=exp_t[:],
                                scalar=1.0,
                                in1=pos_t[:],
                                op0=mybir.AluOpType.bypass,
                                op1=mybir.AluOpType.mult,
                                accum_out=num_parts[:, col:col + 1])

            # epilogue: per (mat, itile) reduce the NJS partials, eps, log
            nm = NMAT * ITILES
            rs_red = accs.tile([P, nm], F32, tag="rsr")
            nc.vector.tensor_reduce(
                out=rs_red[:],
                in_=rs_parts[:].rearrange("p (g j) -> p g j", j=NJS),
                axis=mybir.AxisListType.X, op=mybir.AluOpType.add)
            num_red = accs.tile([P, nm], F32, tag="numr")
            nc.vector.tensor_reduce(
                out=num_red[:],
                in_=num_parts[:].rearrange("p (g j) -> p g j", j=NJS),
                axis=mybir.AxisListType.X, op=mybir.AluOpType.add)
            nc.vector.tensor_scalar_add(out=rs_red[:], in0=rs_red[:],
                                        scalar1=EPS_ROW)
            inv = accs.tile([P, nm], F32, tag="inv")
            nc.vector.reciprocal(inv[:], rs_red[:])
            ratio = accs.tile([P, nm], F32, tag="ratio")
            nc.vector.tensor_tensor(out=ratio[:], in0=num_red[:], in1=inv[:],
                                    op=mybir.AluOpType.mult)
            lg = accs.tile([P, nm], F32, tag="lg")
            loss_acc = accs.tile([P, 1], F32, tag="loss")
            nc.scalar.activation(lg[:], ratio[:],
                                 mybir.ActivationFunctionType.Ln,
                                 accum_out=loss_acc[:])
            nc.sync.dma_start(out.ap(), loss_acc[:])

    _split_excess_waits(nc)
    return nc


# --- host side --------------------------------------------------------------


def _col_perm():
    """Column order produced by the transposed [64, 512] rhs tiles: block b
    position s*128 + p holds global row 64*p + 4*b + s."""
    cp = np.empty(N, dtype=np.int64)
    idx = np.arange(512)
    for b in range(N // 512):
        cp[b * 512 + idx] = 64 * (idx % 128) + 4 * b + idx // 128
    return cp


def _row_perm():
    """Row order of psum partitions: row-tile it, partition p = own row
    8*p + it (from the [64, 128] lhsT tiles)."""
    rp = np.empty(ROWS, dtype=np.int64)
    for it in range(ITILES):
        rp[it * P + np.arange(P)] = ITILES * np.arange(P) + it
    return rp


_NC_CACHE = {}


def make_in_maps(z_mp, z_sc, pos):
    z_mp = np.ascontiguousarray(z_mp, dtype=np.float32)
    z_sc = np.ascontiguousarray(z_sc, dtype=np.float32)
    cp = _col_perm()
    rp = _row_perm()
    ident = np.eye(P, dtype=np.float32)

    pos_cols = np.asarray(pos)[:, cp]
    in_maps = []
    for c in range(NCORES):
        sl = slice(c * ROWS, (c + 1) * ROWS)
        pos_c = np.ascontiguousarray(pos_cols[sl][rp]).astype(np.float16)
        in_maps.append({
            "zmp": z_mp,
            "zsc": z_sc,
            "zmp_own": np.ascontiguousarray(z_mp[sl]),
            "zsc_own": np.ascontiguousarray(z_sc[sl]),
            "pos": pos_c,
            "ident": ident,
        })
    return in_maps


def combine_outputs(outs):
    total = 0.0
    for o in outs:
        total += float(np.asarray(o).astype(np.float64).sum())
    return np.float32(-total / N)


def kernel(z_mp, z_sc, pos):
    from concourse.bass_utils import run_bass_kernel_spmd

    if "nc" not in _NC_CACHE:
        _NC_CACHE["nc"] = build_kernel()
    nc = _NC_CACHE["nc"]

    in_maps = make_in_maps(z_mp, z_sc, pos)
    res = run_bass_kernel_spmd(nc, in_maps, list(range(NCORES)))
    return combine_outputs([res.results[c]["out"] for c in range(NCORES)])
